# revision 7
# baseline (speedup 1.0000x reference)
"""Trainium2 Bass kernel for nn_ContextualModel_75806172774985.

Per-sample computation (B = 4M samples, S=4 steps, Q=5 features):
    y[b, m] = sum_{s < L[b]} q0[b,s] * (A @ feats[b,s])[m],
    A = W_reg @ W_kernel  (4x4)

Sharding: pure data parallel over 8 NeuronCores, batch split 500k/core,
zero-padded to 507904 = 128 partitions x 3968 samples; each partition owns a
contiguous run of samples; tiles of K samples/partition.

v2 design (measured-rate driven):
    GpSimd : mask[s,k] = (L[k] > s)        (TT is_gt, f32)
             z[s,k]   = mask * q0[k,s]     (TT mult, f32)
    VectorE: m_s[k,f] = z[s,k] * x[k,s,f]  (4x TT mult, f32 in -> bf16 out;
             f32r outputs measured 3-6ns/el on DVE, bf16 ~1.3ns/el)
    TensorE: per 512-col group (128 samples/partition):
               4x identity-matmul accumulate (bf16 moving) -> v in PSUM
               4x transpose (bf16) -> vT
               1x block-diag-A matmul (bf16) -> y1[4k''+m, j*128+p] in PSUM
             (no transpose-back: output written in transposed layout, the
             host unscrambles with a pure numpy permutation)
    ScalarE: 3 PSUM->SBUF staging copies per group (casts to bf16)
    Output : bf16, 512B-contiguous DMA runs via paired-chunk DRAM layout
    seq_lengths: one up-front 2MB DMA + one i32->f32 cast
"""
import numpy as np
import ml_dtypes

import concourse.bass as bass
import concourse.tile as tile
from concourse import bacc, mybir
from concourse.bass_utils import run_bass_kernel_spmd

N_CORES = 8
P = 128
B_TOTAL = 4_000_000
BS = B_TOTAL // N_CORES          # 500_000 samples per core

f32 = mybir.dt.float32
bf16 = mybir.dt.bfloat16
i32 = mybir.dt.int32

T = 3968                         # samples per partition (128*T = padded/core)
K_TILES = (256,) * 15 + (128,)
N_GROUPS = T // 128              # 31 groups of 128 samples/partition
Y_ELEMS = N_GROUPS * 2 * 128 * 256   # transposed bf16 output, flat


def build_nc(num_devices=N_CORES):
    assert sum(K_TILES) == T
    bs_pad = P * T

    nc = bacc.Bacc("TRN2", target_bir_lowering=False, debug=False,
                   enable_asserts=False, num_devices=num_devices)

    x_d = nc.dram_tensor("xss", [bs_pad, 20], f32, kind="ExternalInput")
    l_d = nc.dram_tensor("seq", [bs_pad], i32, kind="ExternalInput")
    wk_d = nc.dram_tensor("w_kernel", [4, 4], f32, kind="ExternalInput")
    wr_d = nc.dram_tensor("w_reg", [4, 4], f32, kind="ExternalInput")
    y_d = nc.dram_tensor("y", [Y_ELEMS], bf16, kind="ExternalOutput")

    identb_np = np.eye(128, dtype=np.float32).astype(ml_dtypes.bfloat16)
    identb_d = nc.inline_tensor(identb_np, name="ident128b")
    dmask_np = np.kron(np.eye(32, dtype=np.float32), np.ones((4, 4), np.float32))
    dmask_d = nc.inline_tensor(dmask_np, name="blockdiag_mask")
    siota_np = np.tile(np.arange(4, dtype=np.float32), (128, 1))
    siota_d = nc.inline_tensor(siota_np, name="siota")

    x_flat = x_d.ap().rearrange("(p r) e -> p (r e)", p=P)    # [128, T*20]
    l_flat = l_d.ap().rearrange("(p r) -> p r", p=P)          # [128, T]

    with tile.TileContext(nc) as tc:
        with (
            tc.tile_pool(name="xin", bufs=4) as xin_pool,
            tc.tile_pool(name="mk", bufs=3) as mk_pool,
            tc.tile_pool(name="zp", bufs=3) as z_pool,
            tc.tile_pool(name="g", bufs=16) as g_pool,
            tc.tile_pool(name="vs", bufs=4) as v_pool,
            tc.tile_pool(name="vts", bufs=4) as vt_pool,
            tc.tile_pool(name="yt", bufs=3) as y_pool,
            tc.tile_pool(name="singles", bufs=1) as singles,
            tc.tile_pool(name="ps_v", bufs=3, space="PSUM") as ps_v,
            tc.tile_pool(name="ps_vt", bufs=2, space="PSUM") as ps_vt,
            tc.tile_pool(name="ps_y1", bufs=2, space="PSUM") as ps_y1,
        ):
            # ---- one-time setup ----
            identb = singles.tile([128, 128], bf16)
            nc.sync.dma_start(out=identb[:], in_=identb_d.ap())
            dmask = singles.tile([128, 128], f32)
            nc.sync.dma_start(out=dmask[:], in_=dmask_d.ap())
            siota = singles.tile([128, 4], f32)
            nc.sync.dma_start(out=siota[:], in_=siota_d.ap())
            wk_s = singles.tile([4, 4], f32)
            nc.sync.dma_start(out=wk_s[:], in_=wk_d.ap())            # [c, f]
            wr_s = singles.tile([4, 4], f32)
            nc.sync.dma_start(out=wr_s[:], in_=wr_d.ap().transpose([1, 0]))

            # W_full[4a+f, 4b+m] = sum_c Wk[c,f] * Wreg[m,c] = A[m,f]
            wk_rep = bass.AP(tensor=wk_s.tensor, offset=wk_s.offset,
                             ap=[list(wk_s.ap[0]), [0, 32], [1, 4]])
            wr_rep = bass.AP(tensor=wr_s.tensor, offset=wr_s.offset,
                             ap=[list(wr_s.ap[0]), [0, 32], [1, 4]])
            wkr = singles.tile([4, 128], f32)
            nc.vector.tensor_copy(wkr[:], wk_rep)
            wrr = singles.tile([4, 128], f32)
            nc.vector.tensor_copy(wrr[:], wr_rep)
            wfull_ps = ps_y1.tile([128, 512], f32, tag="y1")
            nc.tensor.matmul(wfull_ps[:, :128], wkr[:], wrr[:])
            w_sb = singles.tile([128, 128], bf16)
            nc.vector.tensor_mul(w_sb[:], wfull_ps[:, :128], dmask[:])

            # seq lengths: one DMA + i32 -> f32 cast
            l_i = singles.tile([P, T], i32)
            nc.sync.dma_start(out=l_i[:], in_=l_flat)
            lf = singles.tile([P, T], f32)
            nc.vector.tensor_copy(lf[:], l_i[:])

            # ---- main loop ----
            base = 0
            for ki, K in enumerate(K_TILES):
                xt = xin_pool.tile([P, K * 20], f32)
                nc.sync.dma_start(out=xt[:],
                                  in_=x_flat[:, base * 20:(base + K) * 20])
                x4 = xt.rearrange("p (k s e) -> p k s e", s=4, e=5)

                # mask[s, k] = (L[k] > s)   (GpSimd, per-s scalar compare)
                mask = mk_pool.tile([P, 4, K], f32, tag="mask")
                for s in range(4):
                    nc.gpsimd.tensor_single_scalar(
                        out=mask[:, s, :], in_=lf[:, base:base + K],
                        scalar=float(s), op=mybir.AluOpType.is_gt)
                # z[s, k] = mask * q0[k, s]  (GpSimd)
                z4 = z_pool.tile([P, 4, K], f32, tag="z")
                q0_sk = bass.AP(tensor=xt.tensor, offset=xt.offset,
                                ap=[list(xt.ap[0]), [5, 4], [20, K]])
                nc.gpsimd.tensor_mul(z4[:], mask[:], q0_sk)

                # m_s[k, f] = z[s, k] * x[k, s, f]  (DVE, bf16 out)
                ms = []
                for s in range(4):
                    m = g_pool.tile([P, K, 4], bf16, tag="m")
                    zb = z4[:, s, :].unsqueeze(2).broadcast_to([P, K, 4])
                    nc.vector.tensor_mul(m[:], zb, x4[:, :, s, 1:5])
                    ms.append(m.rearrange("p k f -> p (k f)"))

                ytile = y_pool.tile([P, K * 4], bf16, tag="y")
                n_groups = (K * 4) // 512
                for g in range(n_groups):
                    sl = slice(g * 512, (g + 1) * 512)
                    # v = sum_s M_s  (PE identity-accumulate, bf16 moving)
                    v_ps = ps_v.tile([128, 512], f32, tag="v")
                    for s in range(4):
                        nc.tensor.matmul(v_ps[:], identb[:], ms[s][:, sl],
                                         start=(s == 0), stop=(s == 3))
                    v_sb = v_pool.tile([128, 512], bf16, tag="v")
                    nc.scalar.copy(v_sb[:], v_ps[:])

                    # vT: transpose each 128-col chunk (bf16)
                    vt_ps = ps_vt.tile([128, 512], bf16, tag="vt")
                    for j in range(4):
                        cj = slice(j * 128, (j + 1) * 128)
                        nc.tensor.transpose(vt_ps[:, cj], v_sb[:, cj], identb[:])
                    vt_sb = vt_pool.tile([128, 512], bf16, tag="vt")
                    nc.scalar.copy(vt_sb[:], vt_ps[:])

                    # y1[4k''+m, j*128+p] = sum_f A[m,f] vT[4k''+f, j*128+p]
                    y1_ps = ps_y1.tile([128, 512], f32, tag="y1")
                    nc.tensor.matmul(y1_ps[:], w_sb[:], vt_sb[:])
                    nc.scalar.copy(ytile[:, sl], y1_ps[:])

                # transposed-layout output DMA: per partition i, runs of
                # 256 contiguous bf16 els (= 512B) per chunk pair
                yd0 = y_d.ap()
                y_ap = bass.AP(tensor=yd0.tensor, offset=yd0.offset + base * 512,
                               ap=[[256, 128], [65536, n_groups],
                                   [32768, 2], [1, 256]])
                nc.sync.dma_start(out=y_ap, in_=ytile[:])
                base += K
    nc.compile()
    return nc, bs_pad


_NC_CACHE = None


def _get_nc():
    global _NC_CACHE
    if _NC_CACHE is None:
        _NC_CACHE = build_nc()
    return _NC_CACHE


def _shard_inputs(xss, seq_lengths, W_kernel, W_reg, bs_pad):
    x2 = np.ascontiguousarray(xss.reshape(B_TOTAL, 20), dtype=np.float32)
    seq = np.ascontiguousarray(seq_lengths, dtype=np.int32)
    wk = np.ascontiguousarray(W_kernel, dtype=np.float32)
    wr = np.ascontiguousarray(W_reg, dtype=np.float32)
    in_maps = []
    pad = bs_pad - BS
    for c in range(N_CORES):
        xs = x2[c * BS:(c + 1) * BS]
        ls = seq[c * BS:(c + 1) * BS]
        if pad:
            xs = np.concatenate([xs, np.zeros((pad, 20), np.float32)], axis=0)
            ls = np.concatenate([ls, np.zeros((pad,), np.int32)], axis=0)
        in_maps.append({"xss": xs, "seq": ls, "w_kernel": wk, "w_reg": wr})
    return in_maps


def _unscramble(y_flat):
    """Invert the transposed DRAM layout -> [bs_pad, 4] f32."""
    a = np.asarray(y_flat).astype(np.float32)
    a6 = a.reshape(N_GROUPS, 2, 32, 4, 2, 128)   # (G, j2, k'', m, jj, p)
    out = np.transpose(a6, (5, 0, 1, 4, 2, 3))   # (p, G, j2, jj, k'', m)
    return np.ascontiguousarray(out).reshape(P * T, 4)


def run(xss, seq_lengths, W_kernel, W_reg, trace=False, **spmd_kwargs):
    nc, bs_pad = _get_nc()
    in_maps = _shard_inputs(xss, seq_lengths, W_kernel, W_reg, bs_pad)
    res = run_bass_kernel_spmd(nc, in_maps, core_ids=list(range(N_CORES)),
                               trace=trace, **spmd_kwargs)
    parts = [_unscramble(r["y"])[:BS] for r in res.results]
    out = np.concatenate(parts, axis=0)
    return out, res


def kernel(xss, seq_lengths, W_kernel, W_reg):
    out, _ = run(xss, seq_lengths, W_kernel, W_reg)
    return out


# revision 13
# speedup vs baseline: 1.7437x; 1.7437x over previous
"""Trainium2 Bass kernel for nn_ContextualModel_75806172774985.

Per-sample computation (B = 4M samples, S=4 steps, Q=5 features):
    y[b, m] = sum_{s < L[b]} q0[b,s] * (A @ feats[b,s])[m],
    A = W_reg @ W_kernel  (4x4)

Sharding: pure data parallel over 8 NeuronCores, batch split 500k/core,
zero-padded to 507904 = 128 partitions x 3968 samples; each partition owns a
contiguous run of samples; tiles of K samples/partition.

v2 design (measured-rate driven):
    GpSimd : mask[s,k] = (L[k] > s)        (TT is_gt, f32)
             z[s,k]   = mask * q0[k,s]     (TT mult, f32)
    VectorE: m_s[k,f] = z[s,k] * x[k,s,f]  (4x TT mult, f32 in -> bf16 out;
             f32r outputs measured 3-6ns/el on DVE, bf16 ~1.3ns/el)
    TensorE: per 512-col group (128 samples/partition):
               4x identity-matmul accumulate (bf16 moving) -> v in PSUM
               4x transpose (bf16) -> vT
               1x block-diag-A matmul (bf16) -> y1[4k''+m, j*128+p] in PSUM
             (no transpose-back: output written in transposed layout, the
             host unscrambles with a pure numpy permutation)
    ScalarE: 3 PSUM->SBUF staging copies per group (casts to bf16)
    Output : bf16, 512B-contiguous DMA runs via paired-chunk DRAM layout
    seq_lengths: one up-front 2MB DMA + one i32->f32 cast
"""
import numpy as np
import ml_dtypes

import concourse.bass as bass
import concourse.tile as tile
from concourse import bacc, mybir
from concourse.bass_utils import run_bass_kernel_spmd

N_CORES = 8
P = 128
B_TOTAL = 4_000_000
BS = B_TOTAL // N_CORES          # 500_000 samples per core

f32 = mybir.dt.float32
bf16 = mybir.dt.bfloat16
i32 = mybir.dt.int32

T = 3968                         # samples per partition (128*T = padded/core)
K_TILES = (256,) * 15 + (128,)
N_GROUPS = T // 128              # 31 groups of 128 samples/partition
Y_ELEMS = N_GROUPS * 2 * 128 * 256   # transposed bf16 output, flat


def build_nc(num_devices=N_CORES):
    assert sum(K_TILES) == T
    bs_pad = P * T

    nc = bacc.Bacc("TRN2", target_bir_lowering=False, debug=False,
                   enable_asserts=False, num_devices=num_devices)

    x_d = nc.dram_tensor("xss", [bs_pad, 20], f32, kind="ExternalInput")
    l_d = nc.dram_tensor("seq", [bs_pad], i32, kind="ExternalInput")
    wk_d = nc.dram_tensor("w_kernel", [4, 4], f32, kind="ExternalInput")
    wr_d = nc.dram_tensor("w_reg", [4, 4], f32, kind="ExternalInput")
    y_d = nc.dram_tensor("y", [Y_ELEMS], bf16, kind="ExternalOutput")

    identb_np = np.eye(128, dtype=np.float32).astype(ml_dtypes.bfloat16)
    identb_d = nc.inline_tensor(identb_np, name="ident128b")
    dmask_np = np.kron(np.eye(32, dtype=np.float32), np.ones((4, 4), np.float32))
    dmask_d = nc.inline_tensor(dmask_np, name="blockdiag_mask")
    x_flat = x_d.ap().rearrange("(p r) e -> p (r e)", p=P)    # [128, T*20]
    l_flat = l_d.ap().rearrange("(p r) -> p r", p=P)          # [128, T]

    with tile.TileContext(nc) as tc:
        with (
            tc.tile_pool(name="xin", bufs=5) as xin_pool,
            tc.tile_pool(name="zp", bufs=3) as z_pool,
            tc.tile_pool(name="g", bufs=16) as g_pool,
            tc.tile_pool(name="vs", bufs=4) as v_pool,
            tc.tile_pool(name="vts", bufs=4) as vt_pool,
            tc.tile_pool(name="yt", bufs=3) as y_pool,
            tc.tile_pool(name="singles", bufs=1) as singles,
            tc.tile_pool(name="ps_v", bufs=3, space="PSUM") as ps_v,
            tc.tile_pool(name="ps_vt", bufs=2, space="PSUM") as ps_vt,
            tc.tile_pool(name="ps_y1", bufs=2, space="PSUM") as ps_y1,
        ):
            # ---- one-time setup ----
            identb = singles.tile([128, 128], bf16)
            nc.sync.dma_start(out=identb[:], in_=identb_d.ap())
            dmask = singles.tile([128, 128], f32)
            nc.sync.dma_start(out=dmask[:], in_=dmask_d.ap())
            wk_s = singles.tile([4, 4], f32)
            nc.sync.dma_start(out=wk_s[:], in_=wk_d.ap())            # [c, f]
            wr_s = singles.tile([4, 4], f32)
            nc.sync.dma_start(out=wr_s[:], in_=wr_d.ap().transpose([1, 0]))

            # W_full[4a+f, 4b+m] = sum_c Wk[c,f] * Wreg[m,c] = A[m,f]
            wk_rep = bass.AP(tensor=wk_s.tensor, offset=wk_s.offset,
                             ap=[list(wk_s.ap[0]), [0, 32], [1, 4]])
            wr_rep = bass.AP(tensor=wr_s.tensor, offset=wr_s.offset,
                             ap=[list(wr_s.ap[0]), [0, 32], [1, 4]])
            wkr = singles.tile([4, 128], f32)
            nc.vector.tensor_copy(wkr[:], wk_rep)
            wrr = singles.tile([4, 128], f32)
            nc.vector.tensor_copy(wrr[:], wr_rep)
            wfull_ps = ps_y1.tile([128, 512], f32, tag="y1")
            nc.tensor.matmul(wfull_ps[:, :128], wkr[:], wrr[:])
            w_sb = singles.tile([128, 128], bf16)
            nc.vector.tensor_mul(w_sb[:], wfull_ps[:, :128], dmask[:])

            # seq lengths: one up-front DMA
            l_i = singles.tile([P, T], i32)
            nc.sync.dma_start(out=l_i[:], in_=l_flat)

            # ---- main loop ----
            base = 0
            for ki, K in enumerate(K_TILES):
                xt = xin_pool.tile([P, K * 20], f32)
                nc.sync.dma_start(out=xt[:],
                                  in_=x_flat[:, base * 20:(base + K) * 20])
                x4 = xt.rearrange("p (k s e) -> p k s e", s=4, e=5)

                # z[s, k] = (L[k] > s) * q0[k, s]   (DVE fused STT)
                z4 = z_pool.tile([P, 4, K], f32, tag="z")
                for s in range(4):
                    nc.vector.scalar_tensor_tensor(
                        out=z4[:, s, :],
                        in0=l_i[:, base:base + K],
                        scalar=float(s),
                        in1=x4[:, :, s, 0],
                        op0=mybir.AluOpType.is_gt,
                        op1=mybir.AluOpType.mult,
                    )

                # m_s[k, f] = z[s, k] * x[k, s, f]  (bf16 out; k-range split
                # DVE/GpSimd by measured rates ~2.2 vs ~3.3 ns/el)
                KA = K * 15 // 32                 # DVE share of samples
                ms = []
                for s in range(4):
                    m = g_pool.tile([P, K, 4], bf16, tag="m")
                    zb = z4[:, s, :].unsqueeze(2).broadcast_to([P, K, 4])
                    nc.vector.tensor_mul(m[:, :KA, :], zb[:, :KA, :],
                                         x4[:, :KA, s, 1:5])
                    nc.gpsimd.tensor_mul(m[:, KA:, :], zb[:, KA:, :],
                                         x4[:, KA:, s, 1:5])
                    ms.append(m.rearrange("p k f -> p (k f)"))

                ytile = y_pool.tile([P, K * 4], bf16, tag="y")
                n_groups = (K * 4) // 512
                for g in range(n_groups):
                    sl = slice(g * 512, (g + 1) * 512)
                    # v = sum_s M_s  (PE identity-accumulate, bf16 moving)
                    v_ps = ps_v.tile([128, 512], f32, tag="v")
                    for s in range(4):
                        nc.tensor.matmul(v_ps[:], identb[:], ms[s][:, sl],
                                         start=(s == 0), stop=(s == 3))
                    v_sb = v_pool.tile([128, 512], bf16, tag="v")
                    nc.scalar.copy(v_sb[:], v_ps[:])

                    # vT: transpose each 128-col chunk (bf16)
                    vt_ps = ps_vt.tile([128, 512], bf16, tag="vt")
                    for j in range(4):
                        cj = slice(j * 128, (j + 1) * 128)
                        nc.tensor.transpose(vt_ps[:, cj], v_sb[:, cj], identb[:])
                    vt_sb = vt_pool.tile([128, 512], bf16, tag="vt")
                    nc.scalar.copy(vt_sb[:], vt_ps[:])

                    # y1[4k''+m, j*128+p] = sum_f A[m,f] vT[4k''+f, j*128+p]
                    y1_ps = ps_y1.tile([128, 512], f32, tag="y1")
                    nc.tensor.matmul(y1_ps[:], w_sb[:], vt_sb[:])
                    nc.scalar.copy(ytile[:, sl], y1_ps[:])

                # transposed-layout output DMA: per partition i, runs of
                # 256 contiguous bf16 els (= 512B) per chunk pair
                yd0 = y_d.ap()
                y_ap = bass.AP(tensor=yd0.tensor, offset=yd0.offset + base * 512,
                               ap=[[256, 128], [65536, n_groups],
                                   [32768, 2], [1, 256]])
                nc.sync.dma_start(out=y_ap, in_=ytile[:])
                base += K
    nc.compile()
    return nc, bs_pad


_NC_CACHE = None


def _get_nc():
    global _NC_CACHE
    if _NC_CACHE is None:
        _NC_CACHE = build_nc()
    return _NC_CACHE


def _shard_inputs(xss, seq_lengths, W_kernel, W_reg, bs_pad):
    x2 = np.ascontiguousarray(xss.reshape(B_TOTAL, 20), dtype=np.float32)
    seq = np.ascontiguousarray(seq_lengths, dtype=np.int32)
    wk = np.ascontiguousarray(W_kernel, dtype=np.float32)
    wr = np.ascontiguousarray(W_reg, dtype=np.float32)
    in_maps = []
    pad = bs_pad - BS
    for c in range(N_CORES):
        xs = x2[c * BS:(c + 1) * BS]
        ls = seq[c * BS:(c + 1) * BS]
        if pad:
            xs = np.concatenate([xs, np.zeros((pad, 20), np.float32)], axis=0)
            ls = np.concatenate([ls, np.zeros((pad,), np.int32)], axis=0)
        in_maps.append({"xss": xs, "seq": ls, "w_kernel": wk, "w_reg": wr})
    return in_maps


def _unscramble(y_flat):
    """Invert the transposed DRAM layout -> [bs_pad, 4] f32."""
    a = np.asarray(y_flat).astype(np.float32)
    a6 = a.reshape(N_GROUPS, 2, 32, 4, 2, 128)   # (G, j2, k'', m, jj, p)
    out = np.transpose(a6, (5, 0, 1, 4, 2, 3))   # (p, G, j2, jj, k'', m)
    return np.ascontiguousarray(out).reshape(P * T, 4)


def run(xss, seq_lengths, W_kernel, W_reg, trace=False, **spmd_kwargs):
    nc, bs_pad = _get_nc()
    in_maps = _shard_inputs(xss, seq_lengths, W_kernel, W_reg, bs_pad)
    res = run_bass_kernel_spmd(nc, in_maps, core_ids=list(range(N_CORES)),
                               trace=trace, **spmd_kwargs)
    parts = [_unscramble(r["y"])[:BS] for r in res.results]
    out = np.concatenate(parts, axis=0)
    return out, res


def kernel(xss, seq_lengths, W_kernel, W_reg):
    out, _ = run(xss, seq_lengths, W_kernel, W_reg)
    return out


# revision 17
# speedup vs baseline: 1.7909x; 1.0271x over previous
"""Trainium2 Bass kernel for nn_ContextualModel_75806172774985.

Per-sample computation (B = 4M samples, S=4 steps, Q=5 features):
    y[b, m] = sum_{s < L[b]} q0[b,s] * (A @ feats[b,s])[m],
    A = W_reg @ W_kernel  (4x4)

Sharding: pure data parallel over 8 NeuronCores, batch split 500k/core,
zero-padded to 507904 = 128 partitions x 3968 samples; each partition owns a
contiguous run of samples; tiles of K samples/partition.

v2 design (measured-rate driven):
    GpSimd : mask[s,k] = (L[k] > s)        (TT is_gt, f32)
             z[s,k]   = mask * q0[k,s]     (TT mult, f32)
    VectorE: m_s[k,f] = z[s,k] * x[k,s,f]  (4x TT mult, f32 in -> bf16 out;
             f32r outputs measured 3-6ns/el on DVE, bf16 ~1.3ns/el)
    TensorE: per 512-col group (128 samples/partition):
               4x identity-matmul accumulate (bf16 moving) -> v in PSUM
               4x transpose (bf16) -> vT
               1x block-diag-A matmul (bf16) -> y1[4k''+m, j*128+p] in PSUM
             (no transpose-back: output written in transposed layout, the
             host unscrambles with a pure numpy permutation)
    ScalarE: 3 PSUM->SBUF staging copies per group (casts to bf16)
    Output : bf16, 512B-contiguous DMA runs via paired-chunk DRAM layout
    seq_lengths: one up-front 2MB DMA + one i32->f32 cast
"""
import numpy as np
import ml_dtypes

import concourse.bass as bass
import concourse.tile as tile
from concourse import bacc, mybir
from concourse.bass_utils import run_bass_kernel_spmd

N_CORES = 8
P = 128
B_TOTAL = 4_000_000
BS = B_TOTAL // N_CORES          # 500_000 samples per core

f32 = mybir.dt.float32
bf16 = mybir.dt.bfloat16
i32 = mybir.dt.int32

T = 3968                         # samples per partition (128*T = padded/core)
K_TILES = (384,) * 10 + (128,)
N_GROUPS = T // 128              # 31 groups of 128 samples/partition
Y_ELEMS = N_GROUPS * 2 * 128 * 256   # transposed bf16 output, flat


def build_nc(num_devices=N_CORES):
    assert sum(K_TILES) == T
    bs_pad = P * T

    nc = bacc.Bacc("TRN2", target_bir_lowering=False, debug=False,
                   enable_asserts=False, num_devices=num_devices)

    x_d = nc.dram_tensor("xss", [bs_pad, 20], f32, kind="ExternalInput")
    l_d = nc.dram_tensor("seq", [bs_pad], i32, kind="ExternalInput")
    wk_d = nc.dram_tensor("w_kernel", [4, 4], f32, kind="ExternalInput")
    wr_d = nc.dram_tensor("w_reg", [4, 4], f32, kind="ExternalInput")
    y_d = nc.dram_tensor("y", [Y_ELEMS], bf16, kind="ExternalOutput")

    identb_np = np.eye(128, dtype=np.float32).astype(ml_dtypes.bfloat16)
    identb_d = nc.inline_tensor(identb_np, name="ident128b")
    dmask_np = np.kron(np.eye(32, dtype=np.float32), np.ones((4, 4), np.float32))
    dmask_d = nc.inline_tensor(dmask_np, name="blockdiag_mask")
    x_flat = x_d.ap().rearrange("(p r) e -> p (r e)", p=P)    # [128, T*20]
    l_flat = l_d.ap().rearrange("(p r) -> p r", p=P)          # [128, T]

    with tile.TileContext(nc) as tc:
        with (
            tc.tile_pool(name="xin", bufs=4) as xin_pool,
            tc.tile_pool(name="zp", bufs=2) as z_pool,
            tc.tile_pool(name="g", bufs=12) as g_pool,
            tc.tile_pool(name="vs", bufs=4) as v_pool,
            tc.tile_pool(name="vts", bufs=4) as vt_pool,
            tc.tile_pool(name="yt", bufs=3) as y_pool,
            tc.tile_pool(name="singles", bufs=1) as singles,
            tc.tile_pool(name="ps_v", bufs=3, space="PSUM") as ps_v,
            tc.tile_pool(name="ps_vt", bufs=2, space="PSUM") as ps_vt,
            tc.tile_pool(name="ps_y1", bufs=2, space="PSUM") as ps_y1,
        ):
            # ---- one-time setup ----
            identb = singles.tile([128, 128], bf16)
            nc.sync.dma_start(out=identb[:], in_=identb_d.ap())
            dmask = singles.tile([128, 128], f32)
            nc.sync.dma_start(out=dmask[:], in_=dmask_d.ap())
            wk_s = singles.tile([4, 4], f32)
            nc.sync.dma_start(out=wk_s[:], in_=wk_d.ap())            # [c, f]
            wr_s = singles.tile([4, 4], f32)
            nc.sync.dma_start(out=wr_s[:], in_=wr_d.ap().transpose([1, 0]))

            # W_full[4a+f, 4b+m] = sum_c Wk[c,f] * Wreg[m,c] = A[m,f]
            wk_rep = bass.AP(tensor=wk_s.tensor, offset=wk_s.offset,
                             ap=[list(wk_s.ap[0]), [0, 32], [1, 4]])
            wr_rep = bass.AP(tensor=wr_s.tensor, offset=wr_s.offset,
                             ap=[list(wr_s.ap[0]), [0, 32], [1, 4]])
            wkr = singles.tile([4, 128], f32)
            nc.vector.tensor_copy(wkr[:], wk_rep)
            wrr = singles.tile([4, 128], f32)
            nc.vector.tensor_copy(wrr[:], wr_rep)
            wfull_ps = ps_y1.tile([128, 512], f32, tag="y1")
            nc.tensor.matmul(wfull_ps[:, :128], wkr[:], wrr[:])
            w_sb = singles.tile([128, 128], bf16)
            nc.vector.tensor_mul(w_sb[:], wfull_ps[:, :128], dmask[:])

            # seq lengths: one up-front DMA
            l_i = singles.tile([P, T], i32)
            nc.sync.dma_start(out=l_i[:], in_=l_flat)

            # ---- main loop ----
            base = 0
            for ki, K in enumerate(K_TILES):
                xt = xin_pool.tile([P, K * 20], f32)
                nc.sync.dma_start(out=xt[:],
                                  in_=x_flat[:, base * 20:(base + K) * 20])
                x4 = xt.rearrange("p (k s e) -> p k s e", s=4, e=5)

                # z[s, k] = (L[k] > s) * q0[k, s]   (DVE fused STT)
                z4 = z_pool.tile([P, 4, K], f32, tag="z")
                for s in range(4):
                    nc.vector.scalar_tensor_tensor(
                        out=z4[:, s, :],
                        in0=l_i[:, base:base + K],
                        scalar=float(s),
                        in1=x4[:, :, s, 0],
                        op0=mybir.AluOpType.is_gt,
                        op1=mybir.AluOpType.mult,
                    )

                # m_s[k, f] = z[s, k] * x[k, s, f]  (bf16 out; k-range split
                # DVE/GpSimd by measured rates ~2.2 vs ~3.3 ns/el)
                KA = K * 13 // 24                 # DVE share of samples
                ms = []
                for s in range(4):
                    m = g_pool.tile([P, K, 4], bf16, tag="m")
                    zb = z4[:, s, :].unsqueeze(2).broadcast_to([P, K, 4])
                    nc.vector.tensor_mul(m[:, :KA, :], zb[:, :KA, :],
                                         x4[:, :KA, s, 1:5])
                    nc.gpsimd.tensor_mul(m[:, KA:, :], zb[:, KA:, :],
                                         x4[:, KA:, s, 1:5])
                    ms.append(m.rearrange("p k f -> p (k f)"))

                ytile = y_pool.tile([P, K * 4], bf16, tag="y")
                n_groups = (K * 4) // 512
                for g in range(n_groups):
                    sl = slice(g * 512, (g + 1) * 512)
                    # v = sum_s M_s  (PE identity-accumulate, bf16 moving)
                    v_ps = ps_v.tile([128, 512], f32, tag="v")
                    for s in range(4):
                        nc.tensor.matmul(v_ps[:], identb[:], ms[s][:, sl],
                                         start=(s == 0), stop=(s == 3))
                    v_sb = v_pool.tile([128, 512], bf16, tag="v")
                    nc.scalar.copy(v_sb[:], v_ps[:])

                    # vT: transpose each 128-col chunk (bf16)
                    vt_ps = ps_vt.tile([128, 512], bf16, tag="vt")
                    for j in range(4):
                        cj = slice(j * 128, (j + 1) * 128)
                        nc.tensor.transpose(vt_ps[:, cj], v_sb[:, cj], identb[:])
                    vt_sb = vt_pool.tile([128, 512], bf16, tag="vt")
                    nc.scalar.copy(vt_sb[:], vt_ps[:])

                    # y1[4k''+m, j*128+p] = sum_f A[m,f] vT[4k''+f, j*128+p]
                    y1_ps = ps_y1.tile([128, 512], f32, tag="y1")
                    nc.tensor.matmul(y1_ps[:], w_sb[:], vt_sb[:])
                    nc.scalar.copy(ytile[:, sl], y1_ps[:])

                # transposed-layout output DMA: per partition i, runs of
                # 256 contiguous bf16 els (= 512B) per chunk pair
                yd0 = y_d.ap()
                y_ap = bass.AP(tensor=yd0.tensor, offset=yd0.offset + base * 512,
                               ap=[[256, 128], [65536, n_groups],
                                   [32768, 2], [1, 256]])
                # issue from the ACT queue: its producer is the ACT engine
                # itself, so this never head-of-line-blocks the x loads on
                # the sync queue
                nc.scalar.dma_start(out=y_ap, in_=ytile[:])
                base += K
    nc.compile()
    return nc, bs_pad


_NC_CACHE = None


def _get_nc():
    global _NC_CACHE
    if _NC_CACHE is None:
        _NC_CACHE = build_nc()
    return _NC_CACHE


def _shard_inputs(xss, seq_lengths, W_kernel, W_reg, bs_pad):
    x2 = np.ascontiguousarray(xss.reshape(B_TOTAL, 20), dtype=np.float32)
    seq = np.ascontiguousarray(seq_lengths, dtype=np.int32)
    wk = np.ascontiguousarray(W_kernel, dtype=np.float32)
    wr = np.ascontiguousarray(W_reg, dtype=np.float32)
    in_maps = []
    pad = bs_pad - BS
    for c in range(N_CORES):
        xs = x2[c * BS:(c + 1) * BS]
        ls = seq[c * BS:(c + 1) * BS]
        if pad:
            xs = np.concatenate([xs, np.zeros((pad, 20), np.float32)], axis=0)
            ls = np.concatenate([ls, np.zeros((pad,), np.int32)], axis=0)
        in_maps.append({"xss": xs, "seq": ls, "w_kernel": wk, "w_reg": wr})
    return in_maps


def _unscramble(y_flat):
    """Invert the transposed DRAM layout -> [bs_pad, 4] f32."""
    a = np.asarray(y_flat).astype(np.float32)
    a6 = a.reshape(N_GROUPS, 2, 32, 4, 2, 128)   # (G, j2, k'', m, jj, p)
    out = np.transpose(a6, (5, 0, 1, 4, 2, 3))   # (p, G, j2, jj, k'', m)
    return np.ascontiguousarray(out).reshape(P * T, 4)


def run(xss, seq_lengths, W_kernel, W_reg, trace=False, **spmd_kwargs):
    nc, bs_pad = _get_nc()
    in_maps = _shard_inputs(xss, seq_lengths, W_kernel, W_reg, bs_pad)
    res = run_bass_kernel_spmd(nc, in_maps, core_ids=list(range(N_CORES)),
                               trace=trace, **spmd_kwargs)
    parts = [_unscramble(r["y"])[:BS] for r in res.results]
    out = np.concatenate(parts, axis=0)
    return out, res


def kernel(xss, seq_lengths, W_kernel, W_reg):
    out, _ = run(xss, seq_lengths, W_kernel, W_reg)
    return out


# revision 19
# speedup vs baseline: 1.9644x; 1.0969x over previous
"""Trainium2 Bass kernel for nn_ContextualModel_75806172774985.

Per-sample computation (B = 4M samples, S=4 steps, Q=5 features):
    y[b, m] = sum_{s < L[b]} q0[b,s] * (A @ feats[b,s])[m],
    A = W_reg @ W_kernel  (4x4)

Sharding: pure data parallel over 8 NeuronCores, batch split 500k/core,
zero-padded to 507904 = 128 partitions x 3968 samples; each partition owns a
contiguous run of samples; tiles of K samples/partition.

v2 design (measured-rate driven):
    GpSimd : mask[s,k] = (L[k] > s)        (TT is_gt, f32)
             z[s,k]   = mask * q0[k,s]     (TT mult, f32)
    VectorE: m_s[k,f] = z[s,k] * x[k,s,f]  (4x TT mult, f32 in -> bf16 out;
             f32r outputs measured 3-6ns/el on DVE, bf16 ~1.3ns/el)
    TensorE: per 512-col group (128 samples/partition):
               4x identity-matmul accumulate (bf16 moving) -> v in PSUM
               4x transpose (bf16) -> vT
               1x block-diag-A matmul (bf16) -> y1[4k''+m, j*128+p] in PSUM
             (no transpose-back: output written in transposed layout, the
             host unscrambles with a pure numpy permutation)
    ScalarE: 3 PSUM->SBUF staging copies per group (casts to bf16)
    Output : bf16, 512B-contiguous DMA runs via paired-chunk DRAM layout
    seq_lengths: one up-front 2MB DMA + one i32->f32 cast
"""
import numpy as np
import ml_dtypes

import concourse.bass as bass
import concourse.tile as tile
from concourse import bacc, mybir
from concourse.bass_utils import run_bass_kernel_spmd

N_CORES = 8
P = 128
B_TOTAL = 4_000_000
BS = B_TOTAL // N_CORES          # 500_000 samples per core

f32 = mybir.dt.float32
bf16 = mybir.dt.bfloat16
i32 = mybir.dt.int32

T = 3968                         # samples per partition (128*T = padded/core)
K_TILES = (384,) * 10 + (128,)
N_GROUPS = T // 128              # 31 groups of 128 samples/partition
Y_ELEMS = N_GROUPS * 2 * 128 * 256   # transposed bf16 output, flat


def build_nc(num_devices=N_CORES):
    assert sum(K_TILES) == T
    bs_pad = P * T

    nc = bacc.Bacc("TRN2", target_bir_lowering=False, debug=False,
                   enable_asserts=False, num_devices=num_devices)

    x_d = nc.dram_tensor("xss", [bs_pad, 20], f32, kind="ExternalInput")
    l_d = nc.dram_tensor("seq", [bs_pad], i32, kind="ExternalInput")
    wk_d = nc.dram_tensor("w_kernel", [4, 4], f32, kind="ExternalInput")
    wr_d = nc.dram_tensor("w_reg", [4, 4], f32, kind="ExternalInput")
    y_d = nc.dram_tensor("y", [Y_ELEMS], bf16, kind="ExternalOutput")

    identb_np = np.eye(128, dtype=np.float32).astype(ml_dtypes.bfloat16)
    identb_d = nc.inline_tensor(identb_np, name="ident128b")
    dmask_np = np.kron(np.eye(32, dtype=np.float32), np.ones((4, 4), np.float32))
    dmask_d = nc.inline_tensor(dmask_np, name="blockdiag_mask")
    x_flat = x_d.ap().rearrange("(p r) e -> p (r e)", p=P)    # [128, T*20]
    l_flat = l_d.ap().rearrange("(p r) -> p r", p=P)          # [128, T]

    with tile.TileContext(nc) as tc:
        with (
            tc.tile_pool(name="xin", bufs=4) as xin_pool,
            tc.tile_pool(name="zp", bufs=2) as z_pool,
            tc.tile_pool(name="g", bufs=12) as g_pool,
            tc.tile_pool(name="vs", bufs=4) as v_pool,
            tc.tile_pool(name="vts", bufs=4) as vt_pool,
            tc.tile_pool(name="yt", bufs=3) as y_pool,
            tc.tile_pool(name="singles", bufs=1) as singles,
            tc.tile_pool(name="ps_v", bufs=3, space="PSUM") as ps_v,
            tc.tile_pool(name="ps_vt", bufs=3, space="PSUM") as ps_vt,
            tc.tile_pool(name="ps_y1", bufs=2, space="PSUM") as ps_y1,
        ):
            # ---- one-time setup ----
            identb = singles.tile([128, 128], bf16)
            nc.sync.dma_start(out=identb[:], in_=identb_d.ap())
            dmask = singles.tile([128, 128], f32)
            nc.sync.dma_start(out=dmask[:], in_=dmask_d.ap())
            wk_s = singles.tile([4, 4], f32)
            nc.sync.dma_start(out=wk_s[:], in_=wk_d.ap())            # [c, f]
            wr_s = singles.tile([4, 4], f32)
            nc.sync.dma_start(out=wr_s[:], in_=wr_d.ap().transpose([1, 0]))

            # W_full[4a+f, 4b+m] = sum_c Wk[c,f] * Wreg[m,c] = A[m,f]
            wk_rep = bass.AP(tensor=wk_s.tensor, offset=wk_s.offset,
                             ap=[list(wk_s.ap[0]), [0, 32], [1, 4]])
            wr_rep = bass.AP(tensor=wr_s.tensor, offset=wr_s.offset,
                             ap=[list(wr_s.ap[0]), [0, 32], [1, 4]])
            wkr = singles.tile([4, 128], f32)
            nc.vector.tensor_copy(wkr[:], wk_rep)
            wrr = singles.tile([4, 128], f32)
            nc.vector.tensor_copy(wrr[:], wr_rep)
            wfull_ps = ps_y1.tile([128, 512], f32, tag="y1")
            nc.tensor.matmul(wfull_ps[:, :128], wkr[:], wrr[:])
            w_sb = singles.tile([128, 128], bf16)
            nc.vector.tensor_mul(w_sb[:], wfull_ps[:, :128], dmask[:])

            # seq lengths: one up-front DMA
            l_i = singles.tile([P, T], i32)
            nc.sync.dma_start(out=l_i[:], in_=l_flat)

            # ---- main loop: software-pipelined group stages ----
            # In slot s the PE runs accum(s), transpose(s-2), y1mm(s-4) and
            # the ACT runs vcopy(s-1), vtcopy(s-3), ycopy(s-5): every
            # instruction's producers finished >= 1 slot earlier, so neither
            # engine queue ever head-of-line blocks.
            jobs = []                 # per group: dict of stage state
            base = 0
            tile_first_job = []
            for K in K_TILES:
                tile_first_job.append(len(jobs))
                n_groups = (K * 4) // 512
                for g in range(n_groups):
                    jobs.append({"tile_base": base, "K": K, "g": g,
                                 "last": g == n_groups - 1})
                base += K

            yd0 = y_d.ap()
            n_jobs = len(jobs)
            for s in range(n_jobs + 5):
                # --- tile-level ops when a tile's first group arrives ---
                if s < n_jobs and s in [tile_first_job[i] for i in range(len(K_TILES))]:
                    ti = tile_first_job.index(s)
                    K = K_TILES[ti]
                    base = jobs[s]["tile_base"]
                    xt = xin_pool.tile([P, K * 20], f32)
                    nc.sync.dma_start(out=xt[:],
                                      in_=x_flat[:, base * 20:(base + K) * 20])
                    x4 = xt.rearrange("p (k s e) -> p k s e", s=4, e=5)

                    # z[s, k] = (L[k] > s) * q0[k, s]   (DVE fused STT)
                    z4 = z_pool.tile([P, 4, K], f32, tag="z")
                    for ss in range(4):
                        nc.vector.scalar_tensor_tensor(
                            out=z4[:, ss, :],
                            in0=l_i[:, base:base + K],
                            scalar=float(ss),
                            in1=x4[:, :, ss, 0],
                            op0=mybir.AluOpType.is_gt,
                            op1=mybir.AluOpType.mult,
                        )
                    # m_s[k, f] = z[s, k] * x[k, s, f]  (bf16 out; k-split
                    # DVE/GpSimd by measured rates ~2.2 vs ~3.3 ns/el)
                    KA = K * 13 // 24
                    ms = []
                    for ss in range(4):
                        m = g_pool.tile([P, K, 4], bf16, tag="m")
                        zb = z4[:, ss, :].unsqueeze(2).broadcast_to([P, K, 4])
                        nc.vector.tensor_mul(m[:, :KA, :], zb[:, :KA, :],
                                             x4[:, :KA, ss, 1:5])
                        nc.gpsimd.tensor_mul(m[:, KA:, :], zb[:, KA:, :],
                                             x4[:, KA:, ss, 1:5])
                        ms.append(m.rearrange("p k f -> p (k f)"))
                    ytile = y_pool.tile([P, K * 4], bf16, tag="y")
                    for j in jobs[s:s + (K * 4) // 512]:
                        j["ms"] = ms
                        j["ytile"] = ytile

                # --- stage accum(s): v = sum_s M_s ---
                if s < n_jobs:
                    jb = jobs[s]
                    sl = slice(jb["g"] * 512, (jb["g"] + 1) * 512)
                    v_ps = ps_v.tile([128, 512], f32, tag="v")
                    for ss in range(4):
                        nc.tensor.matmul(v_ps[:], identb[:], jb["ms"][ss][:, sl],
                                         start=(ss == 0), stop=(ss == 3))
                    jb["v_ps"] = v_ps
                # --- stage vcopy(s-1) ---
                if 0 <= s - 1 < n_jobs:
                    jb = jobs[s - 1]
                    v_sb = v_pool.tile([128, 512], bf16, tag="v")
                    nc.scalar.copy(v_sb[:], jb.pop("v_ps")[:])
                    jb["v_sb"] = v_sb
                # --- stage transpose(s-2) ---
                if 0 <= s - 2 < n_jobs:
                    jb = jobs[s - 2]
                    vt_ps = ps_vt.tile([128, 512], bf16, tag="vt")
                    v_sb = jb.pop("v_sb")
                    for j in range(4):
                        cj = slice(j * 128, (j + 1) * 128)
                        nc.tensor.transpose(vt_ps[:, cj], v_sb[:, cj], identb[:])
                    jb["vt_ps"] = vt_ps
                # --- stage vtcopy(s-3) ---
                if 0 <= s - 3 < n_jobs:
                    jb = jobs[s - 3]
                    vt_sb = vt_pool.tile([128, 512], bf16, tag="vt")
                    nc.scalar.copy(vt_sb[:], jb.pop("vt_ps")[:])
                    jb["vt_sb"] = vt_sb
                # --- stage y1mm(s-4) ---
                if 0 <= s - 4 < n_jobs:
                    jb = jobs[s - 4]
                    y1_ps = ps_y1.tile([128, 512], f32, tag="y1")
                    nc.tensor.matmul(y1_ps[:], w_sb[:], jb.pop("vt_sb")[:])
                    jb["y1_ps"] = y1_ps
                # --- stage ycopy(s-5) + per-tile output DMA ---
                if 0 <= s - 5 < n_jobs:
                    jb = jobs[s - 5]
                    sl = slice(jb["g"] * 512, (jb["g"] + 1) * 512)
                    nc.scalar.copy(jb["ytile"][:, sl], jb.pop("y1_ps")[:])
                    if jb["last"]:
                        K = jb["K"]
                        tb = jb["tile_base"]
                        n_groups = (K * 4) // 512
                        y_ap = bass.AP(tensor=yd0.tensor,
                                       offset=yd0.offset + tb * 512,
                                       ap=[[256, 128], [65536, n_groups],
                                           [32768, 2], [1, 256]])
                        # ACT queue: producer of ytile, so the issue never
                        # head-of-line-blocks the x loads on the sync queue
                        nc.scalar.dma_start(out=y_ap, in_=jb["ytile"][:])
    nc.compile()
    return nc, bs_pad


_NC_CACHE = None


def _get_nc():
    global _NC_CACHE
    if _NC_CACHE is None:
        _NC_CACHE = build_nc()
    return _NC_CACHE


def _shard_inputs(xss, seq_lengths, W_kernel, W_reg, bs_pad):
    x2 = np.ascontiguousarray(xss.reshape(B_TOTAL, 20), dtype=np.float32)
    seq = np.ascontiguousarray(seq_lengths, dtype=np.int32)
    wk = np.ascontiguousarray(W_kernel, dtype=np.float32)
    wr = np.ascontiguousarray(W_reg, dtype=np.float32)
    in_maps = []
    pad = bs_pad - BS
    for c in range(N_CORES):
        xs = x2[c * BS:(c + 1) * BS]
        ls = seq[c * BS:(c + 1) * BS]
        if pad:
            xs = np.concatenate([xs, np.zeros((pad, 20), np.float32)], axis=0)
            ls = np.concatenate([ls, np.zeros((pad,), np.int32)], axis=0)
        in_maps.append({"xss": xs, "seq": ls, "w_kernel": wk, "w_reg": wr})
    return in_maps


def _unscramble(y_flat):
    """Invert the transposed DRAM layout -> [bs_pad, 4] f32."""
    a = np.asarray(y_flat).astype(np.float32)
    a6 = a.reshape(N_GROUPS, 2, 32, 4, 2, 128)   # (G, j2, k'', m, jj, p)
    out = np.transpose(a6, (5, 0, 1, 4, 2, 3))   # (p, G, j2, jj, k'', m)
    return np.ascontiguousarray(out).reshape(P * T, 4)


def run(xss, seq_lengths, W_kernel, W_reg, trace=False, **spmd_kwargs):
    nc, bs_pad = _get_nc()
    in_maps = _shard_inputs(xss, seq_lengths, W_kernel, W_reg, bs_pad)
    res = run_bass_kernel_spmd(nc, in_maps, core_ids=list(range(N_CORES)),
                               trace=trace, **spmd_kwargs)
    parts = [_unscramble(r["y"])[:BS] for r in res.results]
    out = np.concatenate(parts, axis=0)
    return out, res


def kernel(xss, seq_lengths, W_kernel, W_reg):
    out, _ = run(xss, seq_lengths, W_kernel, W_reg)
    return out


# revision 25
# speedup vs baseline: 2.1877x; 1.1137x over previous
"""Trainium2 Bass kernel for nn_ContextualModel_75806172774985.

Per-sample computation (B = 4M samples, S=4 steps, Q=5 features):
    y[b, m] = sum_{s < L[b]} q0[b,s] * (A @ feats[b,s])[m],
    A = W_reg @ W_kernel  (4x4)

Sharding: pure data parallel over 8 NeuronCores, batch split 500k/core,
zero-padded to 507904 = 128 partitions x 3968 samples; each partition owns a
contiguous run of samples; tiles of K samples/partition.

v2 design (measured-rate driven):
    GpSimd : mask[s,k] = (L[k] > s)        (TT is_gt, f32)
             z[s,k]   = mask * q0[k,s]     (TT mult, f32)
    VectorE: m_s[k,f] = z[s,k] * x[k,s,f]  (4x TT mult, f32 in -> bf16 out;
             f32r outputs measured 3-6ns/el on DVE, bf16 ~1.3ns/el)
    TensorE: per 512-col group (128 samples/partition):
               4x identity-matmul accumulate (bf16 moving) -> v in PSUM
               4x transpose (bf16) -> vT
               1x block-diag-A matmul (bf16) -> y1[4k''+m, j*128+p] in PSUM
             (no transpose-back: output written in transposed layout, the
             host unscrambles with a pure numpy permutation)
    ScalarE: 3 PSUM->SBUF staging copies per group (casts to bf16)
    Output : bf16, 512B-contiguous DMA runs via paired-chunk DRAM layout
    seq_lengths: one up-front 2MB DMA + one i32->f32 cast
"""
import numpy as np
import ml_dtypes

import concourse.bass as bass
import concourse.tile as tile
from concourse import bacc, mybir
from concourse.bass_utils import run_bass_kernel_spmd

N_CORES = 8
P = 128
B_TOTAL = 4_000_000
BS = B_TOTAL // N_CORES          # 500_000 samples per core

f32 = mybir.dt.float32
bf16 = mybir.dt.bfloat16
i32 = mybir.dt.int32

T = 3968                         # samples per partition (128*T = padded/core)
K_TILES = (384,) * 10 + (128,)
N_GROUPS = T // 128              # 31 groups of 128 samples/partition
Y_COLS = N_GROUPS * 4 * 128      # per-partition-i output row length
Y_ELEMS = 128 * Y_COLS           # transposed bf16 output, flat


def build_nc(num_devices=N_CORES):
    assert sum(K_TILES) == T
    bs_pad = P * T

    nc = bacc.Bacc("TRN2", target_bir_lowering=False, debug=False,
                   enable_asserts=False, num_devices=num_devices)

    x_d = nc.dram_tensor("xss", [bs_pad, 20], f32, kind="ExternalInput")
    l_d = nc.dram_tensor("seq", [bs_pad], mybir.dt.int8, kind="ExternalInput")
    wk_d = nc.dram_tensor("w_kernel", [4, 4], f32, kind="ExternalInput")
    wr_d = nc.dram_tensor("w_reg", [4, 4], f32, kind="ExternalInput")
    y_d = nc.dram_tensor("y", [Y_ELEMS], bf16, kind="ExternalOutput")

    identb_np = np.eye(128, dtype=np.float32).astype(ml_dtypes.bfloat16)
    identb_d = nc.inline_tensor(identb_np, name="ident128b")
    dmask_np = np.kron(np.eye(32, dtype=np.float32), np.ones((4, 4), np.float32))
    dmask_d = nc.inline_tensor(dmask_np, name="blockdiag_mask")
    x_flat = x_d.ap().rearrange("(p r) e -> p (r e)", p=P)    # [128, T*20]
    l_flat = l_d.ap().rearrange("(p r) -> p r", p=P)          # [128, T]

    with tile.TileContext(nc) as tc:
        with (
            tc.tile_pool(name="xin", bufs=4) as xin_pool,
            tc.tile_pool(name="zp", bufs=2) as z_pool,
            tc.tile_pool(name="g", bufs=12) as g_pool,
            tc.tile_pool(name="vs", bufs=4) as v_pool,
            tc.tile_pool(name="vts", bufs=4) as vt_pool,
            tc.tile_pool(name="yt", bufs=3) as y_pool,
            tc.tile_pool(name="singles", bufs=1) as singles,
            tc.tile_pool(name="ps_v", bufs=3, space="PSUM") as ps_v,
            tc.tile_pool(name="ps_vt", bufs=3, space="PSUM") as ps_vt,
            tc.tile_pool(name="ps_y1", bufs=2, space="PSUM") as ps_y1,
        ):
            # ---- one-time setup ----
            identb = singles.tile([128, 128], bf16)
            nc.sync.dma_start(out=identb[:], in_=identb_d.ap())
            dmask = singles.tile([128, 128], f32)
            nc.sync.dma_start(out=dmask[:], in_=dmask_d.ap())
            wk_s = singles.tile([4, 4], f32)
            nc.sync.dma_start(out=wk_s[:], in_=wk_d.ap())            # [c, f]
            wr_s = singles.tile([4, 4], f32)
            nc.sync.dma_start(out=wr_s[:], in_=wr_d.ap().transpose([1, 0]))

            # W_full[4a+f, 4b+m] = sum_c Wk[c,f] * Wreg[m,c] = A[m,f]
            wk_rep = bass.AP(tensor=wk_s.tensor, offset=wk_s.offset,
                             ap=[list(wk_s.ap[0]), [0, 32], [1, 4]])
            wr_rep = bass.AP(tensor=wr_s.tensor, offset=wr_s.offset,
                             ap=[list(wr_s.ap[0]), [0, 32], [1, 4]])
            wkr = singles.tile([4, 128], f32)
            nc.vector.tensor_copy(wkr[:], wk_rep)
            wrr = singles.tile([4, 128], f32)
            nc.vector.tensor_copy(wrr[:], wr_rep)
            wfull_ps = ps_y1.tile([128, 512], f32, tag="y1")
            nc.tensor.matmul(wfull_ps[:, :128], wkr[:], wrr[:])
            w_sb = singles.tile([128, 128], bf16)
            nc.vector.tensor_mul(w_sb[:], wfull_ps[:, :128], dmask[:])

            # seq lengths: one up-front int8 DMA + cast to f32
            l_i8 = singles.tile([P, T], mybir.dt.int8)
            nc.sync.dma_start(out=l_i8[:], in_=l_flat)
            l_i = singles.tile([P, T], f32)
            nc.vector.tensor_copy(l_i[:], l_i8[:])

            # ---- main loop: software-pipelined group stages ----
            # In slot s the PE runs accum(s), transpose(s-2), y1mm(s-4) and
            # the ACT runs vcopy(s-1), vtcopy(s-3), ycopy(s-5): every
            # instruction's producers finished >= 1 slot earlier, so neither
            # engine queue ever head-of-line blocks.
            jobs = []                 # per group: dict of stage state
            base = 0
            tile_first_job = []
            for K in K_TILES:
                tile_first_job.append(len(jobs))
                n_groups = (K * 4) // 512
                for g in range(n_groups):
                    jobs.append({"tile_base": base, "K": K, "g": g,
                                 "last": g == n_groups - 1})
                base += K

            yd0 = y_d.ap()
            n_jobs = len(jobs)
            for s in range(n_jobs + 5):
                # --- tile-level ops when a tile's first group arrives ---
                if s < n_jobs and s in [tile_first_job[i] for i in range(len(K_TILES))]:
                    ti = tile_first_job.index(s)
                    K = K_TILES[ti]
                    base = jobs[s]["tile_base"]
                    xt = xin_pool.tile([P, K * 20], f32)
                    nc.sync.dma_start(out=xt[:],
                                      in_=x_flat[:, base * 20:(base + K) * 20])
                    x4 = xt.rearrange("p (k s e) -> p k s e", s=4, e=5)

                    # z[s, k] = (L[k] > s) * q0[k, s]   (DVE fused STT)
                    z4 = z_pool.tile([P, 4, K], f32, tag="z")
                    for ss in range(4):
                        nc.vector.scalar_tensor_tensor(
                            out=z4[:, ss, :],
                            in0=l_i[:, base:base + K],
                            scalar=float(ss),
                            in1=x4[:, :, ss, 0],
                            op0=mybir.AluOpType.is_gt,
                            op1=mybir.AluOpType.mult,
                        )
                    # m_s[k, f] = z[s, k] * x[k, s, f]  (bf16 out; k-split
                    # DVE/GpSimd by measured rates ~2.2 vs ~3.3 ns/el)
                    KA = K * 13 // 24
                    ms = []
                    for ss in range(4):
                        m = g_pool.tile([P, K, 4], bf16, tag="m")
                        zb = z4[:, ss, :].unsqueeze(2).broadcast_to([P, K, 4])
                        nc.vector.tensor_mul(m[:, :KA, :], zb[:, :KA, :],
                                             x4[:, :KA, ss, 1:5])
                        nc.gpsimd.tensor_mul(m[:, KA:, :], zb[:, KA:, :],
                                             x4[:, KA:, ss, 1:5])
                        ms.append(m.rearrange("p k f -> p (k f)"))
                    ytile = y_pool.tile([P, K * 4], bf16, tag="y")
                    for j in jobs[s:s + (K * 4) // 512]:
                        j["ms"] = ms
                        j["ytile"] = ytile

                # --- stage accum(s): v = sum_s M_s ---
                if s < n_jobs:
                    jb = jobs[s]
                    sl = slice(jb["g"] * 512, (jb["g"] + 1) * 512)
                    v_ps = ps_v.tile([128, 512], f32, tag="v")
                    for ss in range(4):
                        nc.tensor.matmul(v_ps[:], identb[:], jb["ms"][ss][:, sl],
                                         start=(ss == 0), stop=(ss == 3))
                    jb["v_ps"] = v_ps
                # --- stage vcopy(s-1) ---
                if 0 <= s - 1 < n_jobs:
                    jb = jobs[s - 1]
                    v_sb = v_pool.tile([128, 512], bf16, tag="v")
                    nc.scalar.copy(v_sb[:], jb.pop("v_ps")[:])
                    jb["v_sb"] = v_sb
                # --- stage transpose(s-2) ---
                if 0 <= s - 2 < n_jobs:
                    jb = jobs[s - 2]
                    vt_ps = ps_vt.tile([128, 512], bf16, tag="vt")
                    v_sb = jb.pop("v_sb")
                    for j in range(4):
                        cj = slice(j * 128, (j + 1) * 128)
                        nc.tensor.transpose(vt_ps[:, cj], v_sb[:, cj], identb[:])
                    jb["vt_ps"] = vt_ps
                # --- stage vtcopy(s-3) ---
                if 0 <= s - 3 < n_jobs:
                    jb = jobs[s - 3]
                    vt_sb = vt_pool.tile([128, 512], bf16, tag="vt")
                    nc.scalar.copy(vt_sb[:], jb.pop("vt_ps")[:])
                    jb["vt_sb"] = vt_sb
                # --- stage y1mm(s-4) ---
                if 0 <= s - 4 < n_jobs:
                    jb = jobs[s - 4]
                    y1_ps = ps_y1.tile([128, 512], f32, tag="y1")
                    nc.tensor.matmul(y1_ps[:], w_sb[:], jb.pop("vt_sb")[:])
                    jb["y1_ps"] = y1_ps
                # --- stage ycopy(s-5) + per-tile output DMA ---
                if 0 <= s - 5 < n_jobs:
                    jb = jobs[s - 5]
                    sl = slice(jb["g"] * 512, (jb["g"] + 1) * 512)
                    nc.scalar.copy(jb["ytile"][:, sl], jb.pop("y1_ps")[:])
                    if jb["last"]:
                        K = jb["K"]
                        tb = jb["tile_base"]
                        # partition-major DRAM layout: row i holds all of
                        # partition i's output; one 128-descriptor DMA/tile
                        y_ap = bass.AP(tensor=yd0.tensor,
                                       offset=yd0.offset + tb * 4,
                                       ap=[[Y_COLS, 128], [1, K * 4]])
                        # ACT queue: producer of ytile, so the issue never
                        # head-of-line-blocks the x loads on the sync queue
                        nc.scalar.dma_start(out=y_ap, in_=jb["ytile"][:])
    nc.compile()
    return nc, bs_pad


_NC_CACHE = None


def _get_nc():
    global _NC_CACHE
    if _NC_CACHE is None:
        _NC_CACHE = build_nc()
    return _NC_CACHE


def _shard_inputs(xss, seq_lengths, W_kernel, W_reg, bs_pad):
    x2 = np.ascontiguousarray(xss.reshape(B_TOTAL, 20), dtype=np.float32)
    seq = np.asarray(seq_lengths).astype(np.int8)      # values 0..4, lossless
    wk = np.ascontiguousarray(W_kernel, dtype=np.float32)
    wr = np.ascontiguousarray(W_reg, dtype=np.float32)
    in_maps = []
    pad = bs_pad - BS
    for c in range(N_CORES):
        xs = x2[c * BS:(c + 1) * BS]
        ls = seq[c * BS:(c + 1) * BS]
        if pad:
            xs = np.concatenate([xs, np.zeros((pad, 20), np.float32)], axis=0)
            ls = np.concatenate([ls, np.zeros((pad,), np.int8)], axis=0)
        in_maps.append({"xss": xs, "seq": ls, "w_kernel": wk, "w_reg": wr})
    return in_maps


def _unscramble(y_flat):
    """Invert the transposed DRAM layout -> [bs_pad, 4] f32."""
    a = np.asarray(y_flat).astype(np.float32)
    a5 = a.reshape(32, 4, N_GROUPS, 4, 128)      # (k'', m, G, j, p)
    out = np.transpose(a5, (4, 2, 3, 0, 1))      # (p, G, j, k'', m)
    return np.ascontiguousarray(out).reshape(P * T, 4)


def run(xss, seq_lengths, W_kernel, W_reg, trace=False, **spmd_kwargs):
    nc, bs_pad = _get_nc()
    in_maps = _shard_inputs(xss, seq_lengths, W_kernel, W_reg, bs_pad)
    res = run_bass_kernel_spmd(nc, in_maps, core_ids=list(range(N_CORES)),
                               trace=trace, **spmd_kwargs)
    parts = [_unscramble(r["y"])[:BS] for r in res.results]
    out = np.concatenate(parts, axis=0)
    return out, res


def kernel(xss, seq_lengths, W_kernel, W_reg):
    out, _ = run(xss, seq_lengths, W_kernel, W_reg)
    return out
